# revision 11
# baseline (speedup 1.0000x reference)
import os
"""Bass/Trainium2 kernel for nn_AugmentedTransformer (8-core SPMD, data-parallel over B*D).

Decomposition (validated against the reference in numpy, rel err ~2e-8):
  - head-major channel permutation j' = h*cph + c applied to w_qkv rows,
    w_aug3 rows, w_proj columns, so each head's channels are contiguous.
  - softmax(S_head + wa) factorized as P = exp(wa) * exp(S): E_w = exp(wa)
    is computed once per b (per core), exp(S) per image is tiny [T, H*T].
  - attn apply per channel j: out = (sum_s P*v) / (sum_s P), on DVE with
    bf16 tensor_tensor muls and log2 halving-tree adds (2x mode) for the
    segmented s-reductions; num/den trees merged over one concatenated
    [128, 16384] tile.
Perf structure:
  - program emitted so each in-order engine queue matches execution order
    (E_w chain before qkv; per-image scores/replicate ahead of the DVE chain).
  - PE warmed to 2.4 GHz by a dense matmul burst before the prologue.
  - 4-head-packed qkv matmuls (M=128), tile_position-concurrent scores.
  - exp fused into the scores PSUM->SBUF copy; replicate copies are [128,1024].
  - proj bias-add on ACT, residual-add on GpSimd, flatten DMAs on sync/ACT.
"""
import numpy as np
import ml_dtypes

BF16 = ml_dtypes.bfloat16

# problem constants (hardcoded per contract)
B, D, C, T, TE, H = 2, 32, 256, 64, 1024, 8
CPH = C // H          # 32
G = 32                # groupnorm groups
GSZ = C // G          # 8 channels per group
EPS = 1e-5
NCORES = 8
IMGS = (B * D) // NCORES   # 8 images per core
TT = T * T                 # 4096
NT = IMGS * T              # 512: batched (img, t) free dim
import os as _os
CH1K = 512 if _os.environ.get("PSUM_512") == "1" else 1024
NCHK = TT // CH1K
NHF = CH1K // 512          # 512-wide matmul halves per chunk

_cache = {}


def _build_nc():
    import concourse.bass as bass
    import concourse.mybir as mybir
    from concourse import bacc, tile

    f32 = mybir.dt.float32
    bf16 = mybir.dt.bfloat16
    AF = mybir.ActivationFunctionType
    ALU = mybir.AluOpType

    nc = bacc.Bacc()

    # ---- DRAM I/O ----
    x_d = nc.declare_dram_parameter("x", [IMGS, C, T], f32, isOutput=False)
    temb_d = nc.declare_dram_parameter("temb_bf", [TE, T], bf16, isOutput=False)
    rel3_d = nc.declare_dram_parameter("rel3_aug", [4, TT], bf16, isOutput=False)
    it_d = nc.declare_dram_parameter("it_ind", [T, TT], bf16, isOutput=False)
    w1aT_d = nc.declare_dram_parameter("w1aT", [4, C], bf16, isOutput=False)
    w2T_d = nc.declare_dram_parameter("w2T", [TE, C], bf16, isOutput=False)
    w3T_d = nc.declare_dram_parameter("w3T", [C, C], bf16, isOutput=False)
    wqkvT_d = nc.declare_dram_parameter("wqkvT", [C, 3 * C], bf16, isOutput=False)
    bqkv_d = nc.declare_dram_parameter("bqkv_row", [1, 3 * C], bf16, isOutput=False)
    ones_d = nc.declare_dram_parameter("ones_row", [1, NT], bf16, isOutput=False)
    rep_d = nc.declare_dram_parameter("rep_ind", [H, C], bf16, isOutput=False)
    wprojT_d = nc.declare_dram_parameter("wprojT", [C, C], bf16, isOutput=False)
    gmat_d = nc.declare_dram_parameter("gmat", [128, 16], f32, isOutput=False)
    gmatT_d = nc.declare_dram_parameter("gmatT", [16, 128], f32, isOutput=False)
    aff_d = nc.declare_dram_parameter("aff", [2, 128, 2], f32, isOutput=False)
    b3p_d = nc.declare_dram_parameter("b3p", [2, 128, 1], f32, isOutput=False)
    bproj_d = nc.declare_dram_parameter("bproj", [2, 128, 1], f32, isOutput=False)
    out_d = nc.declare_dram_parameter("out", [IMGS, C, T], f32, isOutput=True)

    with tile.TileContext(nc) as tc:
        with (
            tc.tile_pool(name="const", bufs=1) as constp,
            tc.tile_pool(name="big", bufs=1) as bigp,
            tc.tile_pool(name="work", bufs=2) as workp,
            tc.tile_pool(name="small", bufs=3) as smallp,
            tc.tile_pool(name="pbig", bufs=2, space="PSUM") as pbig,
            tc.tile_pool(name="psmall", bufs=2, space="PSUM") as psmall,
        ):
            # ---- load constants (x + E_w-chain inputs first) ----
            def load(dram, shape, dt, tag):
                t = constp.tile(shape, dt, tag=tag, name=tag)
                nc.sync.dma_start(t[:], dram[:])
                return t

            xt_all = [bigp.tile([128, NT], f32, tag=f"xall{ct}", name=f"xall{ct}") for ct in range(2)]
            for ct in range(2):
                nc.sync.dma_start(
                    xt_all[ct][:],
                    x_d[:, ct * 128:(ct + 1) * 128, :].rearrange("i c t -> c i t"))
            tembt = [constp.tile([128, T], bf16, tag=f"tembt{k}", name=f"tembt{k}") for k in range(8)]
            for k in range(8):
                nc.sync.dma_start(tembt[k][:], temb_d[k * 128:(k + 1) * 128, :])
            w2T = [constp.tile([128, C], bf16, tag=f"w2T{k}", name=f"w2T{k}") for k in range(8)]
            for k in range(8):
                nc.sync.dma_start(w2T[k][:], w2T_d[k * 128:(k + 1) * 128, :])
            rel3 = load(rel3_d, [4, TT], bf16, 'rel3')
            it_ind = load(it_d, [T, TT], bf16, 'itind')
            w1aT = load(w1aT_d, [4, C], bf16, 'w1aT')
            w3T = [constp.tile([128, C], bf16, tag=f"w3T{k}", name=f"w3T{k}") for k in range(2)]
            for k in range(2):
                nc.sync.dma_start(w3T[k][:], w3T_d[k * 128:(k + 1) * 128, :])
            gmat = load(gmat_d, [128, 16], f32, 'gmat')
            gmatT = load(gmatT_d, [16, 128], f32, 'gmatT')
            wqkvT = [constp.tile([128, 3 * C], bf16, tag=f"wqkvT{k}", name=f"wqkvT{k}") for k in range(2)]
            for k in range(2):
                nc.sync.dma_start(wqkvT[k][:], wqkvT_d[k * 128:(k + 1) * 128, :])
            wprojT = [constp.tile([128, C], bf16, tag=f"wprojT{k}", name=f"wprojT{k}") for k in range(2)]
            for k in range(2):
                nc.sync.dma_start(wprojT[k][:], wprojT_d[k * 128:(k + 1) * 128, :])
            bqkv = load(bqkv_d, [1, 3 * C], bf16, 'bqkv')
            ones = load(ones_d, [1, NT], bf16, 'ones')
            repi = load(rep_d, [H, C], bf16, 'repi')
            eps_t = constp.tile([128, 1], f32, name="eps_t")
            nc.gpsimd.memset(eps_t[:], EPS)
            aff = [constp.tile([128, 2], f32, tag=f"aff{k}", name=f"aff{k}") for k in range(2)]
            b3p = [constp.tile([128, 1], f32, tag=f"b3p{k}", name=f"b3p{k}") for k in range(2)]
            bproj = [constp.tile([128, 1], f32, tag=f"bproj{k}", name=f"bproj{k}") for k in range(2)]
            for k in range(2):
                nc.sync.dma_start(aff[k][:], aff_d[k])
                nc.sync.dma_start(b3p[k][:], b3p_d[k])
                nc.sync.dma_start(bproj[k][:], bproj_d[k])

            # big persistent SBUF tensors
            E_w = bigp.tile([128, 2 * TT], bf16, name="Ew")
            PP = bigp.tile([128, 4 * TT], bf16, name="PP")   # [P | P2]
            # relu(emb) aliases into PP (dead once E_w is computed; image-0's
            # P write serializes after the E_w matmul reads — a true dep)
            relu_emb = [PP[:, ot * TT:(ot + 1) * TT] for ot in range(2)]
            T1 = bigp.tile([128, 2 * TT], bf16, name="T1")
            T2 = bigp.tile([128, TT], bf16, name="T2")
            T3 = bigp.tile([128, TT // 2], bf16, name="T3")
            T4 = bigp.tile([128, TT // 4], bf16, name="T4")
            T5 = bigp.tile([128, TT // 8], bf16, name="T5")

            # PE warm-up: ~4.3us of dense back-to-back matmuls at kernel
            # start flips the HAM clock gate to 8/8 (2.4 GHz) before the
            # prologue's real matmul chains run. No data deps — runs while
            # the input DMAs land.
            warm_ps = psmall.tile([1, 512], f32, tag="warm", name="warm_ps", bufs=1)
            warm_t = constp.tile([128, 128], bf16, name="warm_t")
            nc.gpsimd.memset(warm_t[:], 1.0)
            for _ in range(44):
                nc.tensor.matmul(warm_ps[0:1, 0:128], warm_t[:, 0:1],
                                 warm_t[:], start=True, stop=True)

            # ---- phase 1: tp (temb proj), relu(emb), E_w = exp(wa + b3) ----
            tp_ps = pbig.tile([128, CH1K], f32, tag="mm", name="tp")
            for k in range(8):
                nc.tensor.matmul(tp_ps[0:T, 0:C], tembt[k][:], w2T[k][:],
                                 start=(k == 0), stop=(k == 7))
            tpT = constp.tile([T, C], bf16, name="tpT")
            nc.scalar.copy(tpT[:], tp_ps[0:T, 0:C])

            for ot in range(2):
                for chk in range(NCHK):
                    emb_ps = pbig.tile([128, CH1K], f32, tag="mm", name="emb")
                    for hf in range(NHF):
                        sl = slice(chk * CH1K + hf * 512, chk * CH1K + (hf + 1) * 512)
                        psl = slice(hf * 512, (hf + 1) * 512)
                        nc.tensor.matmul(emb_ps[:, psl], w1aT[:, ot * 128:(ot + 1) * 128],
                                         rel3[:, sl], start=True, stop=False)
                        nc.tensor.matmul(emb_ps[:, psl], tpT[:, ot * 128:(ot + 1) * 128],
                                         it_ind[:, sl], start=False, stop=True)
                    osl = slice(chk * CH1K, (chk + 1) * CH1K)
                    nc.vector.tensor_relu(relu_emb[ot][:, osl], emb_ps[:])

            # ---- phase 2a: batched GroupNorm + qkv over all images ----
            ab_all = []
            for ct in range(2):
                stats = smallp.tile([128, 2 * IMGS], f32, tag=f"stats{ct}", name=f"stats{ct}")
                for i in range(IMGS):
                    isl = slice(i * T, (i + 1) * T)
                    sq = workp.tile([128, T], f32, tag="sq", name="sq", bufs=3)
                    nc.scalar.activation(sq[:], xt_all[ct][:, isl], AF.Square,
                                         accum_out=stats[:, IMGS + i:IMGS + i + 1])
                    nc.scalar.activation(sq[:], xt_all[ct][:, isl], AF.Identity,
                                         accum_out=stats[:, i:i + 1])
                gs_ps = pbig.tile([128, CH1K], f32, tag="mm", name="gs")
                nc.tensor.matmul(gs_ps[0:16, 0:2 * IMGS], gmat[:], stats[:],
                                 start=True, stop=True)
                gs = smallp.tile([16, 2 * IMGS], f32, tag="gssb", name="gssb")
                nc.scalar.copy(gs[:], gs_ps[0:16, 0:2 * IMGS])
                cs_ps = pbig.tile([128, CH1K], f32, tag="mm", name="cs")
                nc.tensor.matmul(cs_ps[:, 0:2 * IMGS], gmatT[:], gs[:],
                                 start=True, stop=True)
                cs = smallp.tile([128, 2 * IMGS], f32, tag="cssb", name="cssb")
                nc.scalar.copy(cs[:], cs_ps[:, 0:2 * IMGS])
                # a = rstd*gamma (cols 0:8), cb = beta - mean*a (cols 8:16)
                sc = smallp.tile([128, 3 * IMGS], f32, tag="scn", name="scn")
                inv_n = 1.0 / (GSZ * T)
                nc.vector.tensor_scalar_mul(sc[:, 0:IMGS], cs[:, 0:IMGS], inv_n)
                nc.vector.tensor_scalar_mul(sc[:, IMGS:2 * IMGS], cs[:, IMGS:2 * IMGS], inv_n)
                nc.vector.tensor_tensor(sc[:, 2 * IMGS:], sc[:, 0:IMGS], sc[:, 0:IMGS],
                                        op=ALU.mult)
                var = smallp.tile([128, IMGS], f32, tag="var", name="var")
                nc.vector.tensor_tensor(var[:], sc[:, IMGS:2 * IMGS], sc[:, 2 * IMGS:],
                                        op=ALU.subtract)
                std = smallp.tile([128, IMGS], f32, tag="std", name="std")
                nc.scalar.activation(std[:], var[:], AF.Sqrt, bias=eps_t[:])
                rstd = smallp.tile([128, IMGS], f32, tag="rstd", name="rstd")
                nc.vector.reciprocal(rstd[:], std[:])
                abt = smallp.tile([128, 2 * IMGS], f32, tag=f"ab{ct}", name=f"ab{ct}")
                gam = aff[ct][:, 0:1].broadcast_to([128, IMGS])
                bet = aff[ct][:, 1:2].broadcast_to([128, IMGS])
                nc.vector.tensor_tensor(abt[:, 0:IMGS], rstd[:], gam, op=ALU.mult)
                tmp = smallp.tile([128, IMGS], f32, tag="tmpn", name="tmpn")
                nc.vector.tensor_tensor(tmp[:], sc[:, 0:IMGS], abt[:, 0:IMGS], op=ALU.mult)
                nc.vector.tensor_tensor(abt[:, IMGS:], bet, tmp[:], op=ALU.subtract)
                ab_all.append(abt)

            hbf = [bigp.tile([128, NT], bf16, tag=f"hall{ct}", name=f"hall{ct}") for ct in range(2)]
            for ct in range(2):
                for i in range(IMGS):
                    isl = slice(i * T, (i + 1) * T)
                    nc.scalar.activation(hbf[ct][:, isl], xt_all[ct][:, isl], AF.Identity,
                                         scale=ab_all[ct][:, i:i + 1],
                                         bias=ab_all[ct][:, IMGS + i:IMGS + i + 1])


            for jt in range(2):
                for chk in range(NCHK):
                    wa_ps = pbig.tile([128, CH1K], f32, tag="mm", name="wa")
                    for hf in range(NHF):
                        sl = slice(chk * CH1K + hf * 512, chk * CH1K + (hf + 1) * 512)
                        psl = slice(hf * 512, (hf + 1) * 512)
                        for it in range(2):
                            nc.tensor.matmul(wa_ps[:, psl], w3T[it][:, jt * 128:(jt + 1) * 128],
                                             relu_emb[it][:, sl],
                                             start=(it == 0), stop=(it == 1))
                    osl = slice(jt * TT + chk * CH1K, jt * TT + (chk + 1) * CH1K)
                    nc.scalar.activation(E_w[:, osl], wa_ps[:], AF.Exp, bias=b3p[jt][:])


            # qkv batched per head (M=32, N=512): scores operands must sit
            # at partition base 0 (codegen rejects offset stationary tiles),
            # so each head gets its own [32, NT] q/k tile. PSUM evacuation on
            # the (prologue-idle) Vector engine.
            q4, k4 = [], []
            for g in range(H):
                for ofs, lst, nm in ((0, q4, "q"), (C, k4, "k")):
                    ps = pbig.tile([128, CH1K], f32, tag="mm", name=f"{nm}p{g}")
                    mo = ofs + g * 32
                    for it in range(2):
                        nc.tensor.matmul(ps[0:32, 0:NT], wqkvT[it][:, mo:mo + 32],
                                         hbf[it][:], start=(it == 0), stop=False)
                    nc.tensor.matmul(ps[0:32, 0:NT], bqkv[:, mo:mo + 32], ones[:],
                                     start=False, stop=True)
                    t = constp.tile([32, NT], bf16, name=f"{nm}4{g}")
                    nc.vector.tensor_copy(t[:], ps[0:32, 0:NT])
                    lst.append(t)
            v_cat = constp.tile([128, 2 * NT], bf16, name="vcat")
            for m in range(2):
                ps = pbig.tile([128, CH1K], f32, tag="mm", name="vps")
                mo = 2 * C + m * 128
                for it in range(2):
                    nc.tensor.matmul(ps[:, 0:NT], wqkvT[it][:, mo:mo + 128],
                                     hbf[it][:], start=(it == 0), stop=False)
                nc.tensor.matmul(ps[:, 0:NT], bqkv[:, mo:mo + 128], ones[:],
                                 start=False, stop=True)
                nc.vector.tensor_copy(v_cat[:, m * NT:(m + 1) * NT], ps[:, 0:NT])

            # ---- phase 2b: per-image attention apply ----
            hv_q = []        # (hv_cat tile, image) pending projection

            def emit_proj(hv, ip):
                ipsl = slice(ip * T, (ip + 1) * T)
                proj_ps = psmall.tile([128, 2 * T], f32, tag="proj", name="proj",
                                      bufs=1)
                for m in range(2):
                    for jt in range(2):
                        nc.tensor.matmul(proj_ps[:, m * T:(m + 1) * T],
                                         wprojT[jt][:, m * 128:(m + 1) * 128],
                                         hv[:, jt * T:(jt + 1) * T],
                                         start=(jt == 0), stop=(jt == 1))
                for m in range(2):
                    tmpo = workp.tile([128, T], f32, tag=f"to{m}", name=f"to{m}")
                    nc.scalar.activation(tmpo[:], proj_ps[:, m * T:(m + 1) * T],
                                         AF.Identity, bias=bproj[m][:])
                    osb = workp.tile([128, T], f32, tag=f"o{m}", name=f"o{m}")
                    if os.environ.get("NO_GPS") == "1":
                        nc.vector.tensor_tensor(osb[:], tmpo[:], xt_all[m][:, ipsl],
                                                op=ALU.add)
                    else:
                        nc.gpsimd.tensor_add(osb[:], tmpo[:], xt_all[m][:, ipsl])
                    nc.gpsimd.dma_start(out_d[ip, m * 128:(m + 1) * 128, :], osb[:])

            for i in range(IMGS):
                isl = slice(i * T, (i + 1) * T)

                # scores per head -> [T, H*T]; 4 concurrent via tile_position
                s_ps = psmall.tile([T, H * T], f32, tag="sc", name="scores", bufs=2)
                for h in range(H):
                    nc.tensor.matmul(s_ps[:, h * T:(h + 1) * T],
                                     q4[h][:, isl], k4[h][:, isl],
                                     start=True, stop=True)
                # exp fused into the PSUM->SBUF evacuation
                ssb = workp.tile([T, H * T], bf16, tag="ssb", name="ssb")
                nc.scalar.activation(ssb[:], s_ps[:], AF.Exp)

                # flatten to head-major [H, T*T] (DMA partition gather)
                s_hm = workp.tile([H, TT], bf16, tag="shm", name="shm")
                for h in range(H):
                    nc.sync.dma_start(s_hm[h:h + 1, :], ssb[:, h * T:(h + 1) * T])

                # replicate exp(S) to all 128 channel rows via PE one-hot
                esr = workp.tile([128, 2 * TT], bf16, tag="esr", name="esr")
                for jt in range(2):
                    for chk in range(NCHK):
                        rep_ps = pbig.tile([128, CH1K], f32, tag="mm", name="rep")
                        for hf in range(NHF):
                            sl = slice(jt * TT + chk * CH1K + hf * 512,
                                       jt * TT + chk * CH1K + (hf + 1) * 512)
                            shsl = slice(chk * CH1K + hf * 512,
                                         chk * CH1K + (hf + 1) * 512)
                            nc.tensor.matmul(rep_ps[:, hf * 512:(hf + 1) * 512],
                                             repi[:, jt * 128:(jt + 1) * 128],
                                             s_hm[:, shsl], start=True, stop=True)
                        osl = slice(jt * TT + chk * CH1K, jt * TT + (chk + 1) * CH1K)
                        nc.scalar.copy(esr[:, osl], rep_ps[:])

                if hv_q:
                    emit_proj(*hv_q.pop(0))

                # ---- DVE chain ----
                # P = E_w * esr
                nc.vector.tensor_tensor(PP[:, 0:2 * TT], E_w[:], esr[:], op=ALU.mult)
                # P2 = P * v (v broadcast along t)
                for jt in range(2):
                    src = PP[:, jt * TT:(jt + 1) * TT]
                    dst = PP[:, 2 * TT + jt * TT:2 * TT + (jt + 1) * TT]
                    vb = v_cat[:, jt * NT + i * T:jt * NT + (i + 1) * T]
                    vb = vb.unsqueeze(1).broadcast_to([128, T, T])
                    nc.vector.tensor_tensor(dst.rearrange("p (t s) -> p t s", s=T),
                                            src.rearrange("p (t s) -> p t s", s=T),
                                            vb, op=ALU.mult)
                # halving-tree sum over s for [den | num] in one pass
                lvl = [(PP, T1, 32), (T1, T2, 16), (T2, T3, 8), (T3, T4, 4), (T4, T5, 2)]
                for src_t, dst_t, w in lvl:
                    c4 = src_t[:].rearrange("p (g two w) -> p g two w", two=2, w=w)
                    nc.vector.tensor_tensor(dst_t[:].rearrange("p (g w) -> p g w", w=w),
                                            c4[:, :, 0, :], c4[:, :, 1, :], op=ALU.add)
                sums = smallp.tile([128, 4 * T], f32, tag="sums", name="sums")
                c4 = T5[:].rearrange("p (g two w) -> p g two w", two=2, w=1)
                nc.vector.tensor_tensor(sums[:].rearrange("p (g w) -> p g w", w=1),
                                        c4[:, :, 0, :], c4[:, :, 1, :], op=ALU.add)
                rec = smallp.tile([128, 2 * T], f32, tag="rec", name="rec")
                if os.environ.get("NO_APPROX") == "1":
                    nc.vector.reciprocal(rec[:], sums[:, 0:2 * T])
                else:
                    nc.vector.reciprocal_approx_fast(rec[:], sums[:, 0:2 * T])
                hv = workp.tile([128, 2 * T], bf16, tag="hv", name="hv")
                nc.vector.tensor_tensor(hv[:], sums[:, 2 * T:4 * T], rec[:],
                                        op=ALU.mult)
                hv_q.append((hv, i))

            for hv, ip in hv_q:
                emit_proj(hv, ip)

    nc.compile()
    return nc


def _host_prep(inputs):
    x = np.ascontiguousarray(inputs["x"], np.float32)
    temb = np.asarray(inputs["temb"], np.float32)
    fi = np.asarray(inputs["frame_indices"]).astype(np.int64)
    w_qkv = np.asarray(inputs["w_qkv"], np.float32)
    b_qkv = np.asarray(inputs["b_qkv"], np.float32)
    w_aug1 = np.asarray(inputs["w_aug1"], np.float32)
    b_aug1 = np.asarray(inputs["b_aug1"], np.float32)
    w_aug2 = np.asarray(inputs["w_aug2"], np.float32)
    b_aug2 = np.asarray(inputs["b_aug2"], np.float32)
    w_aug3 = np.asarray(inputs["w_aug3"], np.float32)
    b_aug3 = np.asarray(inputs["b_aug3"], np.float32)
    w_proj = np.asarray(inputs["w_proj"], np.float32)
    b_proj = np.asarray(inputs["b_proj"], np.float32)
    gamma = np.asarray(inputs["norm_scale"], np.float32)
    beta = np.asarray(inputs["norm_bias"], np.float32)

    jp = np.arange(C)
    perm = (jp % CPH) * H + jp // CPH   # perm[j'] = old j
    scale2 = np.float32(1.0 / np.sqrt(CPH))

    wq = w_qkv[0 * C:1 * C][perm] * scale2
    wk = w_qkv[1 * C:2 * C][perm]
    wv = w_qkv[2 * C:3 * C][perm]
    w_qkv_p = np.concatenate([wq, wk, wv], 0)
    b_qkv_p = np.concatenate([b_qkv[0 * C:C][perm] * scale2,
                              b_qkv[C:2 * C][perm], b_qkv[2 * C:][perm]], 0)

    rel = fi[:, None, :] - fi[:, :, None]
    rel3 = np.stack([np.clip(rel, 0, None), np.clip(-rel, 0, None),
                     (rel == 0)], 1).astype(np.float32)
    rel3 = np.log1p(rel3).reshape(B, 3, TT)
    rel3_aug = np.concatenate([rel3, np.ones((B, 1, TT), np.float32)], 1)
    w1a = np.concatenate([w_aug1, (b_aug1 + b_aug2)[:, None]], 1)  # [C, 4]

    it_ind = np.zeros((T, TT), np.float32)
    tsel = np.repeat(np.arange(T), T)
    it_ind[tsel, np.arange(TT)] = 1.0

    rep_ind = np.zeros((H, C), np.float32)
    rep_ind[np.repeat(np.arange(H), CPH), np.arange(C)] = 1.0

    gmat = np.zeros((128, 16), np.float32)
    gmat[np.arange(128), np.arange(128) // GSZ] = 1.0
    gmatT = np.ascontiguousarray(gmat.T)

    aff = np.stack([gamma.reshape(2, 128), beta.reshape(2, 128)], -1)  # [2,128,2]
    b3p = b_aug3[perm].reshape(2, 128, 1)
    bproj = b_proj.reshape(2, 128, 1)

    common = {
        "it_ind": it_ind.astype(BF16),
        "w1aT": np.ascontiguousarray(w1a.T).astype(BF16),
        "w2T": np.ascontiguousarray(w_aug2.T).astype(BF16),
        "w3T": np.ascontiguousarray(w_aug3[perm].T).astype(BF16),
        "wqkvT": np.ascontiguousarray(w_qkv_p.T).astype(BF16),
        "bqkv_row": b_qkv_p.reshape(1, 3 * C).astype(BF16),
        "ones_row": np.ones((1, NT), BF16),
        "rep_ind": rep_ind.astype(BF16),
        "wprojT": np.ascontiguousarray(w_proj[:, perm].T).astype(BF16),
        "gmat": gmat, "gmatT": gmatT,
        "aff": np.ascontiguousarray(aff),
        "b3p": np.ascontiguousarray(b3p),
        "bproj": np.ascontiguousarray(bproj),
    }
    xr = x.reshape(B * D, C, T)
    in_maps = []
    for core in range(NCORES):
        b = (core * IMGS) // D
        m = dict(common)
        m["x"] = np.ascontiguousarray(xr[core * IMGS:(core + 1) * IMGS])
        m["temb_bf"] = temb[b].astype(BF16)
        m["rel3_aug"] = rel3_aug[b].astype(BF16)
        in_maps.append(m)
    return in_maps


def kernel(**inputs):
    from concourse.bass_utils import run_bass_kernel_spmd

    if "nc" not in _cache:
        _cache["nc"] = _build_nc()
    nc = _cache["nc"]
    in_maps = _host_prep(inputs)
    res = run_bass_kernel_spmd(nc, in_maps, core_ids=list(range(NCORES)))
    outs = [np.asarray(res.results[i]["out"]) for i in range(NCORES)]
    full = np.concatenate(outs, 0).reshape(B, D, C, T)
    return full.astype(np.float32)


# revision 13
# speedup vs baseline: 1.1227x; 1.1227x over previous
import os
"""Bass/Trainium2 kernel for nn_AugmentedTransformer (8-core SPMD, data-parallel over B*D).

Decomposition (validated against the reference in numpy, rel err ~2e-8):
  - head-major channel permutation j' = h*cph + c applied to w_qkv rows,
    w_aug3 rows, w_proj columns, so each head's channels are contiguous.
  - softmax(S_head + wa) factorized as P = exp(wa) * exp(S): E_w = exp(wa)
    is computed once per b (per core), exp(S) per image is tiny [T, H*T].
  - attn apply per channel j: out = (sum_s P*v) / (sum_s P), on DVE with
    bf16 tensor_tensor muls and log2 halving-tree adds (2x mode) for the
    segmented s-reductions; num/den trees merged over one concatenated
    [128, 16384] tile.
Perf structure:
  - program emitted so each in-order engine queue matches execution order
    (E_w chain before qkv; per-image scores/replicate ahead of the DVE chain).
  - PE warmed to 2.4 GHz by a dense matmul burst before the prologue.
  - 4-head-packed qkv matmuls (M=128), tile_position-concurrent scores.
  - exp fused into the scores PSUM->SBUF copy; replicate copies are [128,1024].
  - proj bias-add on ACT, residual-add on GpSimd, flatten DMAs on sync/ACT.
"""
import numpy as np
import ml_dtypes

BF16 = ml_dtypes.bfloat16

# problem constants (hardcoded per contract)
B, D, C, T, TE, H = 2, 32, 256, 64, 1024, 8
CPH = C // H          # 32
G = 32                # groupnorm groups
GSZ = C // G          # 8 channels per group
EPS = 1e-5
NCORES = 8
IMGS = (B * D) // NCORES   # 8 images per core
TT = T * T                 # 4096
NT = IMGS * T              # 512: batched (img, t) free dim
import os as _os
CH1K = 512 if _os.environ.get("PSUM_512") == "1" else 1024
NCHK = TT // CH1K
NHF = CH1K // 512          # 512-wide matmul halves per chunk

_cache = {}


def _build_nc():
    import concourse.bass as bass
    import concourse.mybir as mybir
    from concourse import bacc, tile

    f32 = mybir.dt.float32
    bf16 = mybir.dt.bfloat16
    AF = mybir.ActivationFunctionType
    ALU = mybir.AluOpType

    nc = bacc.Bacc()

    # ---- DRAM I/O ----
    x_d = nc.declare_dram_parameter("x", [IMGS, C, T], f32, isOutput=False)
    temb_d = nc.declare_dram_parameter("temb_bf", [TE, T], bf16, isOutput=False)
    rel3_d = nc.declare_dram_parameter("rel3_aug", [4, TT], bf16, isOutput=False)
    it_d = nc.declare_dram_parameter("it_ind", [T, TT], bf16, isOutput=False)
    w1aT_d = nc.declare_dram_parameter("w1aT", [4, C], bf16, isOutput=False)
    w2T_d = nc.declare_dram_parameter("w2T", [TE, C], bf16, isOutput=False)
    w3T_d = nc.declare_dram_parameter("w3T", [C, C], bf16, isOutput=False)
    wqkvT_d = nc.declare_dram_parameter("wqkvT", [C, 3 * C], bf16, isOutput=False)
    bqkv_d = nc.declare_dram_parameter("bqkv_row", [1, 3 * C], bf16, isOutput=False)
    ones_d = nc.declare_dram_parameter("ones_row", [1, NT], bf16, isOutput=False)
    rep_d = nc.declare_dram_parameter("rep_ind", [64, 16 * 128], bf16, isOutput=False)
    wprojT_d = nc.declare_dram_parameter("wprojT", [C, C], bf16, isOutput=False)
    gmat_d = nc.declare_dram_parameter("gmat", [128, 16], f32, isOutput=False)
    gmatT_d = nc.declare_dram_parameter("gmatT", [16, 128], f32, isOutput=False)
    aff_d = nc.declare_dram_parameter("aff", [2, 128, 2], f32, isOutput=False)
    b3p_d = nc.declare_dram_parameter("b3p", [2, 128, 1], f32, isOutput=False)
    bproj_d = nc.declare_dram_parameter("bproj", [2, 128, 1], f32, isOutput=False)
    out_d = nc.declare_dram_parameter("out", [IMGS, C, T], f32, isOutput=True)

    with tile.TileContext(nc) as tc:
        with (
            tc.tile_pool(name="const", bufs=1) as constp,
            tc.tile_pool(name="big", bufs=1) as bigp,
            tc.tile_pool(name="work", bufs=2) as workp,
            tc.tile_pool(name="small", bufs=3) as smallp,
            tc.tile_pool(name="pbig", bufs=2, space="PSUM") as pbig,
            tc.tile_pool(name="psmall", bufs=2, space="PSUM") as psmall,
        ):
            # ---- load constants (x + E_w-chain inputs first) ----
            def load(dram, shape, dt, tag):
                t = constp.tile(shape, dt, tag=tag, name=tag)
                nc.sync.dma_start(t[:], dram[:])
                return t

            xt_all = [bigp.tile([128, NT], f32, tag=f"xall{ct}", name=f"xall{ct}") for ct in range(2)]
            for ct in range(2):
                nc.sync.dma_start(
                    xt_all[ct][:],
                    x_d[:, ct * 128:(ct + 1) * 128, :].rearrange("i c t -> c i t"))
            tembt = [constp.tile([128, T], bf16, tag=f"tembt{k}", name=f"tembt{k}") for k in range(8)]
            for k in range(8):
                nc.sync.dma_start(tembt[k][:], temb_d[k * 128:(k + 1) * 128, :])
            w2T = [constp.tile([128, C], bf16, tag=f"w2T{k}", name=f"w2T{k}") for k in range(8)]
            for k in range(8):
                nc.sync.dma_start(w2T[k][:], w2T_d[k * 128:(k + 1) * 128, :])
            rel3 = load(rel3_d, [4, TT], bf16, 'rel3')
            it_ind = load(it_d, [T, TT], bf16, 'itind')
            w1aT = load(w1aT_d, [4, C], bf16, 'w1aT')
            w3T = [constp.tile([128, C], bf16, tag=f"w3T{k}", name=f"w3T{k}") for k in range(2)]
            for k in range(2):
                nc.sync.dma_start(w3T[k][:], w3T_d[k * 128:(k + 1) * 128, :])
            gmat = load(gmat_d, [128, 16], f32, 'gmat')
            gmatT = load(gmatT_d, [16, 128], f32, 'gmatT')
            wqkvT = [constp.tile([128, 3 * C], bf16, tag=f"wqkvT{k}", name=f"wqkvT{k}") for k in range(2)]
            for k in range(2):
                nc.sync.dma_start(wqkvT[k][:], wqkvT_d[k * 128:(k + 1) * 128, :])
            wprojT = [constp.tile([128, C], bf16, tag=f"wprojT{k}", name=f"wprojT{k}") for k in range(2)]
            for k in range(2):
                nc.sync.dma_start(wprojT[k][:], wprojT_d[k * 128:(k + 1) * 128, :])
            bqkv = load(bqkv_d, [1, 3 * C], bf16, 'bqkv')
            ones = load(ones_d, [1, NT], bf16, 'ones')
            repi2 = load(rep_d, [64, 16 * 128], bf16, 'repi2')
            eps_t = constp.tile([128, 1], f32, name="eps_t")
            nc.gpsimd.memset(eps_t[:], EPS)
            aff = [constp.tile([128, 2], f32, tag=f"aff{k}", name=f"aff{k}") for k in range(2)]
            b3p = [constp.tile([128, 1], f32, tag=f"b3p{k}", name=f"b3p{k}") for k in range(2)]
            bproj = [constp.tile([128, 1], f32, tag=f"bproj{k}", name=f"bproj{k}") for k in range(2)]
            for k in range(2):
                nc.sync.dma_start(aff[k][:], aff_d[k])
                nc.sync.dma_start(b3p[k][:], b3p_d[k])
                nc.sync.dma_start(bproj[k][:], bproj_d[k])

            # big persistent SBUF tensors
            E_w = bigp.tile([128, 2 * TT], bf16, name="Ew")
            PP = bigp.tile([128, 4 * TT], bf16, name="PP")   # [P | P2]
            # relu(emb) aliases into PP (dead once E_w is computed; image-0's
            # P write serializes after the E_w matmul reads — a true dep)
            relu_emb = [PP[:, ot * TT:(ot + 1) * TT] for ot in range(2)]
            T1 = bigp.tile([128, 2 * TT], bf16, name="T1")
            T2 = bigp.tile([128, TT], bf16, name="T2")
            T3 = bigp.tile([128, TT // 2], bf16, name="T3")
            T4 = bigp.tile([128, TT // 4], bf16, name="T4")
            T5 = bigp.tile([128, TT // 8], bf16, name="T5")

            # PE warm-up: ~4.3us of dense back-to-back matmuls at kernel
            # start flips the HAM clock gate to 8/8 (2.4 GHz) before the
            # prologue's real matmul chains run. No data deps — runs while
            # the input DMAs land.
            warm_ps = psmall.tile([1, 512], f32, tag="warm", name="warm_ps", bufs=1)
            warm_t = constp.tile([128, 128], bf16, name="warm_t")
            nc.gpsimd.memset(warm_t[:], 1.0)
            for _ in range(44):
                nc.tensor.matmul(warm_ps[0:1, 0:128], warm_t[:, 0:1],
                                 warm_t[:], start=True, stop=True)

            # ---- phase 1: tp (temb proj), relu(emb), E_w = exp(wa + b3) ----
            tp_ps = pbig.tile([128, CH1K], f32, tag="mm", name="tp")
            for k in range(8):
                nc.tensor.matmul(tp_ps[0:T, 0:C], tembt[k][:], w2T[k][:],
                                 start=(k == 0), stop=(k == 7))
            tpT = constp.tile([T, C], bf16, name="tpT")
            nc.scalar.copy(tpT[:], tp_ps[0:T, 0:C])

            for ot in range(2):
                for chk in range(NCHK):
                    emb_ps = pbig.tile([128, CH1K], f32, tag="mm", name="emb")
                    for hf in range(NHF):
                        sl = slice(chk * CH1K + hf * 512, chk * CH1K + (hf + 1) * 512)
                        psl = slice(hf * 512, (hf + 1) * 512)
                        nc.tensor.matmul(emb_ps[:, psl], w1aT[:, ot * 128:(ot + 1) * 128],
                                         rel3[:, sl], start=True, stop=False)
                        nc.tensor.matmul(emb_ps[:, psl], tpT[:, ot * 128:(ot + 1) * 128],
                                         it_ind[:, sl], start=False, stop=True)
                    osl = slice(chk * CH1K, (chk + 1) * CH1K)
                    nc.vector.tensor_relu(relu_emb[ot][:, osl], emb_ps[:])

            # ---- phase 2a: batched GroupNorm + qkv over all images ----
            ab_all = []
            for ct in range(2):
                stats = smallp.tile([128, 2 * IMGS], f32, tag=f"stats{ct}", name=f"stats{ct}")
                for i in range(IMGS):
                    isl = slice(i * T, (i + 1) * T)
                    sq = workp.tile([128, T], f32, tag="sq", name="sq", bufs=3)
                    nc.scalar.activation(sq[:], xt_all[ct][:, isl], AF.Square,
                                         accum_out=stats[:, IMGS + i:IMGS + i + 1])
                    nc.scalar.activation(sq[:], xt_all[ct][:, isl], AF.Identity,
                                         accum_out=stats[:, i:i + 1])
                gs_ps = pbig.tile([128, CH1K], f32, tag="mm", name="gs")
                nc.tensor.matmul(gs_ps[0:16, 0:2 * IMGS], gmat[:], stats[:],
                                 start=True, stop=True)
                gs = smallp.tile([16, 2 * IMGS], f32, tag="gssb", name="gssb")
                nc.scalar.copy(gs[:], gs_ps[0:16, 0:2 * IMGS])
                cs_ps = pbig.tile([128, CH1K], f32, tag="mm", name="cs")
                nc.tensor.matmul(cs_ps[:, 0:2 * IMGS], gmatT[:], gs[:],
                                 start=True, stop=True)
                cs = smallp.tile([128, 2 * IMGS], f32, tag="cssb", name="cssb")
                nc.scalar.copy(cs[:], cs_ps[:, 0:2 * IMGS])
                # a = rstd*gamma (cols 0:8), cb = beta - mean*a (cols 8:16)
                sc = smallp.tile([128, 3 * IMGS], f32, tag="scn", name="scn")
                inv_n = 1.0 / (GSZ * T)
                nc.vector.tensor_scalar_mul(sc[:, 0:IMGS], cs[:, 0:IMGS], inv_n)
                nc.vector.tensor_scalar_mul(sc[:, IMGS:2 * IMGS], cs[:, IMGS:2 * IMGS], inv_n)
                nc.vector.tensor_tensor(sc[:, 2 * IMGS:], sc[:, 0:IMGS], sc[:, 0:IMGS],
                                        op=ALU.mult)
                var = smallp.tile([128, IMGS], f32, tag="var", name="var")
                nc.vector.tensor_tensor(var[:], sc[:, IMGS:2 * IMGS], sc[:, 2 * IMGS:],
                                        op=ALU.subtract)
                std = smallp.tile([128, IMGS], f32, tag="std", name="std")
                nc.scalar.activation(std[:], var[:], AF.Sqrt, bias=eps_t[:])
                rstd = smallp.tile([128, IMGS], f32, tag="rstd", name="rstd")
                nc.vector.reciprocal(rstd[:], std[:])
                abt = smallp.tile([128, 2 * IMGS], f32, tag=f"ab{ct}", name=f"ab{ct}")
                gam = aff[ct][:, 0:1].broadcast_to([128, IMGS])
                bet = aff[ct][:, 1:2].broadcast_to([128, IMGS])
                nc.vector.tensor_tensor(abt[:, 0:IMGS], rstd[:], gam, op=ALU.mult)
                tmp = smallp.tile([128, IMGS], f32, tag="tmpn", name="tmpn")
                nc.vector.tensor_tensor(tmp[:], sc[:, 0:IMGS], abt[:, 0:IMGS], op=ALU.mult)
                nc.vector.tensor_tensor(abt[:, IMGS:], bet, tmp[:], op=ALU.subtract)
                ab_all.append(abt)

            hbf = [bigp.tile([128, NT], bf16, tag=f"hall{ct}", name=f"hall{ct}") for ct in range(2)]
            for ct in range(2):
                for i in range(IMGS):
                    isl = slice(i * T, (i + 1) * T)
                    nc.scalar.activation(hbf[ct][:, isl], xt_all[ct][:, isl], AF.Identity,
                                         scale=ab_all[ct][:, i:i + 1],
                                         bias=ab_all[ct][:, IMGS + i:IMGS + i + 1])


            for jt in range(2):
                for chk in range(NCHK):
                    wa_ps = pbig.tile([128, CH1K], f32, tag="mm", name="wa")
                    for hf in range(NHF):
                        sl = slice(chk * CH1K + hf * 512, chk * CH1K + (hf + 1) * 512)
                        psl = slice(hf * 512, (hf + 1) * 512)
                        for it in range(2):
                            nc.tensor.matmul(wa_ps[:, psl], w3T[it][:, jt * 128:(jt + 1) * 128],
                                             relu_emb[it][:, sl],
                                             start=(it == 0), stop=(it == 1))
                    osl = slice(jt * TT + chk * CH1K, jt * TT + (chk + 1) * CH1K)
                    nc.scalar.activation(E_w[:, osl], wa_ps[:], AF.Exp, bias=b3p[jt][:])


            # qkv batched per head (M=32, N=512): scores operands must sit
            # at partition base 0 (codegen rejects offset stationary tiles),
            # so each head gets its own [32, NT] q/k tile. PSUM evacuation on
            # the (prologue-idle) Vector engine.
            q4, k4 = [], []
            for g in range(H):
                for ofs, lst, nm in ((0, q4, "q"), (C, k4, "k")):
                    ps = pbig.tile([128, CH1K], f32, tag="mm", name=f"{nm}p{g}")
                    mo = ofs + g * 32
                    for it in range(2):
                        nc.tensor.matmul(ps[0:32, 0:NT], wqkvT[it][:, mo:mo + 32],
                                         hbf[it][:], start=(it == 0), stop=False)
                    nc.tensor.matmul(ps[0:32, 0:NT], bqkv[:, mo:mo + 32], ones[:],
                                     start=False, stop=True)
                    t = constp.tile([32, NT], bf16, name=f"{nm}4{g}")
                    nc.vector.tensor_copy(t[:], ps[0:32, 0:NT])
                    lst.append(t)
            v_cat = constp.tile([128, 2 * NT], bf16, name="vcat")
            for m in range(2):
                ps = pbig.tile([128, CH1K], f32, tag="mm", name="vps")
                mo = 2 * C + m * 128
                for it in range(2):
                    nc.tensor.matmul(ps[:, 0:NT], wqkvT[it][:, mo:mo + 128],
                                     hbf[it][:], start=(it == 0), stop=False)
                nc.tensor.matmul(ps[:, 0:NT], bqkv[:, mo:mo + 128], ones[:],
                                 start=False, stop=True)
                nc.vector.tensor_copy(v_cat[:, m * NT:(m + 1) * NT], ps[:, 0:NT])

            # ---- phase 2b-pre: scores + exp + flatten for ALL images ----
            # Hoisted out of the per-image cycle so the main loop's only
            # cross-engine chain is PE-replicate -> ACT-evacuate -> DVE.
            s_hm_all = bigp.tile([64, TT], bf16, name="shmall")
            for i in range(IMGS):
                isl = slice(i * T, (i + 1) * T)
                s_ps = psmall.tile([T, H * T], f32, tag="sc", name="scores", bufs=2)
                for h in range(H):
                    nc.tensor.matmul(s_ps[:, h * T:(h + 1) * T],
                                     q4[h][:, isl], k4[h][:, isl],
                                     start=True, stop=True)
                ssb = workp.tile([T, H * T], bf16, tag="ssb", name="ssb")
                nc.scalar.activation(ssb[:], s_ps[:], AF.Exp)
                for h in range(H):
                    nc.sync.dma_start(s_hm_all[i * 8 + h:i * 8 + h + 1, :],
                                      ssb[:, h * T:(h + 1) * T])

            # ---- phase 2b: per-image attention apply ----
            hv_q = []        # (hv tile, image) pending projection

            def emit_proj(hv, ip):
                ipsl = slice(ip * T, (ip + 1) * T)
                proj_ps = psmall.tile([128, 2 * T], f32, tag="proj", name="proj",
                                      bufs=1)
                for m in range(2):
                    for jt in range(2):
                        nc.tensor.matmul(proj_ps[:, m * T:(m + 1) * T],
                                         wprojT[jt][:, m * 128:(m + 1) * 128],
                                         hv[:, jt * T:(jt + 1) * T],
                                         start=(jt == 0), stop=(jt == 1))
                for m in range(2):
                    tmpo = workp.tile([128, T], f32, tag=f"to{m}", name=f"to{m}")
                    nc.scalar.activation(tmpo[:], proj_ps[:, m * T:(m + 1) * T],
                                         AF.Identity, bias=bproj[m][:])
                    osb = workp.tile([128, T], f32, tag=f"o{m}", name=f"o{m}")
                    if os.environ.get("NO_GPS") == "1":
                        nc.vector.tensor_tensor(osb[:], tmpo[:], xt_all[m][:, ipsl],
                                                op=ALU.add)
                    else:
                        nc.gpsimd.tensor_add(osb[:], tmpo[:], xt_all[m][:, ipsl])
                    nc.gpsimd.dma_start(out_d[ip, m * 128:(m + 1) * 128, :], osb[:])

            for i in range(IMGS):
                # replicate exp(S) of image i to all 128 channel rows (K=64
                # one-hot selector picks this image's 8 head rows)
                esr = workp.tile([128, 2 * TT], bf16, tag="esr", name="esr")
                for jt in range(2):
                    for chk in range(NCHK):
                        rep_ps = pbig.tile([128, CH1K], f32, tag="mm", name="rep")
                        rsl = slice((2 * i + jt) * 128, (2 * i + jt + 1) * 128)
                        for hf in range(NHF):
                            shsl = slice(chk * CH1K + hf * 512,
                                         chk * CH1K + (hf + 1) * 512)
                            nc.tensor.matmul(rep_ps[:, hf * 512:(hf + 1) * 512],
                                             repi2[:, rsl],
                                             s_hm_all[:, shsl], start=True, stop=True)
                        osl = slice(jt * TT + chk * CH1K, jt * TT + (chk + 1) * CH1K)
                        nc.scalar.copy(esr[:, osl], rep_ps[:])

                if hv_q:
                    emit_proj(*hv_q.pop(0))

                # ---- DVE chain ----
                nc.vector.tensor_tensor(PP[:, 0:2 * TT], E_w[:], esr[:], op=ALU.mult)
                for jt in range(2):
                    psrc = PP[:, jt * TT:(jt + 1) * TT]
                    pdst = PP[:, 2 * TT + jt * TT:2 * TT + (jt + 1) * TT]
                    vb = v_cat[:, jt * NT + i * T:jt * NT + (i + 1) * T]
                    vb = vb.unsqueeze(1).broadcast_to([128, T, T])
                    nc.vector.tensor_tensor(pdst.rearrange("p (t s) -> p t s", s=T),
                                            psrc.rearrange("p (t s) -> p t s", s=T),
                                            vb, op=ALU.mult)
                # halving-tree sum over s for [den | num]
                lvl = [(PP, T1, 32), (T1, T2, 16), (T2, T3, 8), (T3, T4, 4), (T4, T5, 2)]
                for src_t, dst_t, w in lvl:
                    c4 = src_t[:].rearrange("p (g two w) -> p g two w", two=2, w=w)
                    nc.vector.tensor_tensor(dst_t[:].rearrange("p (g w) -> p g w", w=w),
                                            c4[:, :, 0, :], c4[:, :, 1, :], op=ALU.add)
                sums = smallp.tile([128, 4 * T], f32, tag="sums", name="sums")
                c4 = T5[:].rearrange("p (g two w) -> p g two w", two=2, w=1)
                nc.vector.tensor_tensor(sums[:].rearrange("p (g w) -> p g w", w=1),
                                        c4[:, :, 0, :], c4[:, :, 1, :], op=ALU.add)
                rec = smallp.tile([128, 2 * T], f32, tag="rec", name="rec")
                if os.environ.get("NO_APPROX") == "1":
                    nc.vector.reciprocal(rec[:], sums[:, 0:2 * T])
                else:
                    nc.vector.reciprocal_approx_fast(rec[:], sums[:, 0:2 * T])
                hv = workp.tile([128, 2 * T], bf16, tag="hv", name="hv")
                nc.vector.tensor_tensor(hv[:], sums[:, 2 * T:4 * T], rec[:],
                                        op=ALU.mult)
                hv_q.append((hv, i))

            for hv, ip in hv_q:
                emit_proj(hv, ip)

    nc.compile()
    return nc


def _host_prep(inputs):
    x = np.ascontiguousarray(inputs["x"], np.float32)
    temb = np.asarray(inputs["temb"], np.float32)
    fi = np.asarray(inputs["frame_indices"]).astype(np.int64)
    w_qkv = np.asarray(inputs["w_qkv"], np.float32)
    b_qkv = np.asarray(inputs["b_qkv"], np.float32)
    w_aug1 = np.asarray(inputs["w_aug1"], np.float32)
    b_aug1 = np.asarray(inputs["b_aug1"], np.float32)
    w_aug2 = np.asarray(inputs["w_aug2"], np.float32)
    b_aug2 = np.asarray(inputs["b_aug2"], np.float32)
    w_aug3 = np.asarray(inputs["w_aug3"], np.float32)
    b_aug3 = np.asarray(inputs["b_aug3"], np.float32)
    w_proj = np.asarray(inputs["w_proj"], np.float32)
    b_proj = np.asarray(inputs["b_proj"], np.float32)
    gamma = np.asarray(inputs["norm_scale"], np.float32)
    beta = np.asarray(inputs["norm_bias"], np.float32)

    jp = np.arange(C)
    perm = (jp % CPH) * H + jp // CPH   # perm[j'] = old j
    scale2 = np.float32(1.0 / np.sqrt(CPH))

    wq = w_qkv[0 * C:1 * C][perm] * scale2
    wk = w_qkv[1 * C:2 * C][perm]
    wv = w_qkv[2 * C:3 * C][perm]
    w_qkv_p = np.concatenate([wq, wk, wv], 0)
    b_qkv_p = np.concatenate([b_qkv[0 * C:C][perm] * scale2,
                              b_qkv[C:2 * C][perm], b_qkv[2 * C:][perm]], 0)

    rel = fi[:, None, :] - fi[:, :, None]
    rel3 = np.stack([np.clip(rel, 0, None), np.clip(-rel, 0, None),
                     (rel == 0)], 1).astype(np.float32)
    rel3 = np.log1p(rel3).reshape(B, 3, TT)
    rel3_aug = np.concatenate([rel3, np.ones((B, 1, TT), np.float32)], 1)
    w1a = np.concatenate([w_aug1, (b_aug1 + b_aug2)[:, None]], 1)  # [C, 4]

    it_ind = np.zeros((T, TT), np.float32)
    tsel = np.repeat(np.arange(T), T)
    it_ind[tsel, np.arange(TT)] = 1.0

    # replicate selector: for (image i, channel-half jt), slice
    # [:, (2i+jt)*128:(2i+jt+1)*128] is [64, 128] with row i*8 + jt*4 + j//32
    # hot for output channel j — copies exp(S) head rows to channel rows.
    rep_ind = np.zeros((64, 16 * 128), np.float32)
    for i in range(IMGS):
        for jt in range(2):
            col0 = (2 * i + jt) * 128
            for j in range(128):
                h = jt * 4 + j // CPH
                rep_ind[i * 8 + h, col0 + j] = 1.0

    gmat = np.zeros((128, 16), np.float32)
    gmat[np.arange(128), np.arange(128) // GSZ] = 1.0
    gmatT = np.ascontiguousarray(gmat.T)

    aff = np.stack([gamma.reshape(2, 128), beta.reshape(2, 128)], -1)  # [2,128,2]
    b3p = b_aug3[perm].reshape(2, 128, 1)
    bproj = b_proj.reshape(2, 128, 1)

    common = {
        "it_ind": it_ind.astype(BF16),
        "w1aT": np.ascontiguousarray(w1a.T).astype(BF16),
        "w2T": np.ascontiguousarray(w_aug2.T).astype(BF16),
        "w3T": np.ascontiguousarray(w_aug3[perm].T).astype(BF16),
        "wqkvT": np.ascontiguousarray(w_qkv_p.T).astype(BF16),
        "bqkv_row": b_qkv_p.reshape(1, 3 * C).astype(BF16),
        "ones_row": np.ones((1, NT), BF16),
        "rep_ind": rep_ind.astype(BF16),
        "wprojT": np.ascontiguousarray(w_proj[:, perm].T).astype(BF16),
        "gmat": gmat, "gmatT": gmatT,
        "aff": np.ascontiguousarray(aff),
        "b3p": np.ascontiguousarray(b3p),
        "bproj": np.ascontiguousarray(bproj),
    }
    xr = x.reshape(B * D, C, T)
    in_maps = []
    for core in range(NCORES):
        b = (core * IMGS) // D
        m = dict(common)
        m["x"] = np.ascontiguousarray(xr[core * IMGS:(core + 1) * IMGS])
        m["temb_bf"] = temb[b].astype(BF16)
        m["rel3_aug"] = rel3_aug[b].astype(BF16)
        in_maps.append(m)
    return in_maps


def kernel(**inputs):
    from concourse.bass_utils import run_bass_kernel_spmd

    if "nc" not in _cache:
        _cache["nc"] = _build_nc()
    nc = _cache["nc"]
    in_maps = _host_prep(inputs)
    res = run_bass_kernel_spmd(nc, in_maps, core_ids=list(range(NCORES)))
    outs = [np.asarray(res.results[i]["out"]) for i in range(NCORES)]
    full = np.concatenate(outs, 0).reshape(B, D, C, T)
    return full.astype(np.float32)
